# revision 17
# baseline (speedup 1.0000x reference)
"""Trainium2 Bass kernel for nn_ConvAttLIF (conv3x3 + temporal attention + LIF scan).

Sharding: data-parallel over batch B=16 across 8 NeuronCores (2 samples/core).

Layout: frames host-packed with shared row halos (33-wide rows: the right
halo of row r is the left halo of row r+1, both zero), so a frame is 1124
contiguous cols and the conv output span is 1056 cols = 3 psum chunks of 352.

Conv: per chunk, 15 f32r matmuls accumulate one psum bank:
  - 3 "pair" units (K=128): taps (-1,dx) and (+1,dx) fused by storing a
    second frame copy shifted 2 rows (66 cols) in partitions 64-127.
  - 3 "single" units (K=64): taps (0,dx) on partitions 0-63.
  - 9 "corr" units (K=128): [x_hi; x_lo] . [w_lo; w_hi] per tap, restoring
    ~fp32 accuracy from the 12-bit f32r operands (x_hi = trunc13(x)).
Chunks are processed in rotating order (frame f starts at chunk f%3) so each
frame's first psum bank was drained one chunk-stream earlier - no PE stall.

LIF scan: attention folded in via v_t = u_t/att_t, so each step is
v = g*c_t + y (STT), g = (v < thr_t)*v (STT, same engine - no cross-engine
hop in the serial chain), spike = (v >= thr_t) off-chain. The sample-1 tail
(no conv left to overlap) splits rows across DVE/Pool/ACT.

kernel(**inputs) takes the FULL unsharded inputs, returns the FULL output.
"""
import sys

sys.path.insert(0, "/opt/trn_rl_repo")

import numpy as np
import ml_dtypes
import concourse.bass as bass
import concourse.bacc as bacc
import concourse.tile as tile
import concourse.mybir as mybir
from concourse.bass_utils import run_bass_kernel_spmd

F32 = mybir.dt.float32
F32R = mybir.dt.float32r
FP8 = mybir.dt.float8e4
BF16 = mybir.dt.bfloat16
DR = mybir.MatmulPerfMode.DoubleRow
AF = mybir.ActivationFunctionType
OP = mybir.AluOpType
AX = mybir.AxisListType

B, T, CIN, H, W = 16, 20, 64, 32, 32
CH = 128
N_CORES = 8
BPC = B // N_CORES
ALPHA, VTH = 0.3, 0.6
HW = H * W                     # 1024
PW = W + 1                     # 33: row stride (shared halo col)
XCOL = 34 * PW + 2             # 1124 packed frame cols (+2 guard)
MAR = 2 * PW                   # 66: left margin in XA for the shifted copy
CN = 352                       # psum chunk cols (3 x 352 = 1056 out span)
OUT0 = PW + 1                  # 34: first out position in frame coords
NY = 25                        # y-tile ring size
TAPS = [(dy, dx) for dy in (-1, 0, 1) for dx in (-1, 0, 1)]


def _build_program():
    nc = bacc.Bacc("TRN2", target_bir_lowering=False, debug=False,
                   num_devices=N_CORES)

    xhi_d = nc.dram_tensor("xhi", [BPC, T, CIN, XCOL], F32,
                           kind="ExternalInput").ap()
    xc66_d = nc.dram_tensor("xc66", [BPC, T, 128, 2, MAR + XCOL], FP8,
                            kind="ExternalInput").ap()
    xc2_d = nc.dram_tensor("xc2", [BPC, T, 128, 2, 2 + XCOL], FP8,
                           kind="ExternalInput").ap()
    wpair_d = nc.dram_tensor("wpair", [128, 3 * 128], F32,
                             kind="ExternalInput").ap()
    wsing_d = nc.dram_tensor("wsing", [64, 3 * 128], F32,
                             kind="ExternalInput").ap()
    wc66_d = nc.dram_tensor("wc66", [128, 2, 3 * 128], FP8,
                            kind="ExternalInput").ap()
    wc2_d = nc.dram_tensor("wc2", [128, 2, 128], FP8,
                           kind="ExternalInput").ap()
    wc0_d = nc.dram_tensor("wc0", [128, 2, 128], FP8,
                           kind="ExternalInput").ap()
    bias_d = nc.dram_tensor("bias", [128, 1], F32, kind="ExternalInput").ap()
    w1t_d = nc.dram_tensor("w1t", [T, 5], F32, kind="ExternalInput").ap()
    w2t_d = nc.dram_tensor("w2t", [5, T], F32, kind="ExternalInput").ap()
    ident_d = nc.dram_tensor("ident", [128, 128], F32, kind="ExternalInput").ap()
    spk = nc.dram_tensor("spk", [BPC, T, CH, HW], FP8,
                         kind="ExternalOutput").ap()

    with tile.TileContext(nc) as tc:
        with tc.tile_pool(name="sb", bufs=1) as P1, \
             tc.tile_pool(name="scr", bufs=2) as P2, \
             tc.tile_pool(name="so", bufs=3) as P3, \
             tc.tile_pool(name="ps", bufs=1, space="PSUM") as PP:

            # ---- persistent tiles ----
            xas = [P1.tile([128, MAR + XCOL], F32R, tag=f"xa{i}", name=f"xa{i}")
                   for i in range(4)]
            xc66s = [P1.tile([128, 2, MAR + XCOL], FP8, tag=f"x6{i}",
                             name=f"x6{i}") for i in range(4)]
            xc2s = [P1.tile([128, 2, 2 + XCOL], FP8, tag=f"x2{i}",
                            name=f"x2{i}") for i in range(4)]

            def x_dma(s, t):
                f = s * T + t
                src = xhi_d[s, t].bitcast(F32R)
                nc.sync.dma_start(xas[f % 4][0:64, MAR:MAR + XCOL], src)
                nc.sync.dma_start(xas[f % 4][64:128, 0:XCOL], src)
                nc.sync.dma_start(xc66s[f % 4][:], xc66_d[s, t])
                nc.sync.dma_start(xc2s[f % 4][:], xc2_d[s, t])

            # startup order: frame-0 XA halves, pair/single weights (first
            # units of the first chunk), then the corr inputs
            f0src = xhi_d[0, 0].bitcast(F32R)
            nc.sync.dma_start(xas[0][0:64, MAR:MAR + XCOL], f0src)
            wsing = P1.tile([64, 3 * 128], F32R, tag="wsing", name="wsing")
            nc.sync.dma_start(wsing[:], wsing_d[:].bitcast(F32R))
            nc.sync.dma_start(xas[0][64:128, 0:XCOL], f0src)
            wpair = P1.tile([128, 3 * 128], F32R, tag="wpair", name="wpair")
            nc.sync.dma_start(wpair[:], wpair_d[:].bitcast(F32R))
            nc.sync.dma_start(xc66s[0][:], xc66_d[0, 0])
            nc.sync.dma_start(xc2s[0][:], xc2_d[0, 0])
            bias_t = P1.tile([128, 1], F32, tag="bias", name="bias")
            nc.sync.dma_start(bias_t[:], bias_d[:])

            wc66_s = P1.tile([128, 2, 3 * 128], FP8, tag="wc66", name="wc66")
            nc.sync.dma_start(wc66_s[:], wc66_d[:])
            wc2_s = P1.tile([128, 2, 128], FP8, tag="wc2", name="wc2")
            nc.sync.dma_start(wc2_s[:], wc2_d[:])
            wc0_s = P1.tile([128, 2, 128], FP8, tag="wc0", name="wc0")
            nc.sync.dma_start(wc0_s[:], wc0_d[:])
            w1t_s = P1.tile([T, 5], F32, tag="w1t", name="w1t")
            nc.sync.dma_start(w1t_s[:], w1t_d[:])
            w2t_s = P1.tile([5, T], F32, tag="w2t", name="w2t")
            nc.sync.dma_start(w2t_s[:], w2t_d[:])
            ident = P1.tile([128, 128], F32, tag="ident", name="ident")
            nc.sync.dma_start(ident[:], ident_d[:])
            ones_t = P1.tile([1, 128], F32, tag="ones", name="ones")
            nc.vector.memset(ones_t[:], 1.0)

            ys = [P1.tile([128, XCOL], F32, tag=f"y{i}", name=f"y{i}")
                  for i in range(NY)]
            gs = [P1.tile([128, HW], F32, tag=f"g{s}", name=f"g{s}")
                  for s in range(BPC)]
            # stats rows: 0-2 chunk sums, 3 -junk, 4 total, 5 max
            s_st = [P1.tile([128, 6 * T], F32, tag=f"S{s}", name=f"S{s}")
                    for s in range(BPC)]
            bc = [P1.tile([128, 4 * T], F32, tag=f"bc{s}", name=f"bc{s}")
                  for s in range(BPC)]

            engines = {"v": nc.vector, "p": nc.gpsimd}

            def conv_frame(s, t, skip_dma=False):
                f = s * T + t
                if not skip_dma:
                    x_dma(s, t)
                xa, x6, x2 = xas[f % 4], xc66s[f % 4], xc2s[f % 4]
                y = ys[f % NY]
                for ci in range(3):
                    c = (f + ci) % 3
                    o = OUT0 + CN * c
                    ps = PP.tile([128, CN], F32, tag=f"p{c}{f % 2}",
                                 name=f"p{c}{f % 2}")
                    units = []
                    for i, dx in enumerate((-1, 0, 1)):
                        units.append((wsing[:, i * 128:(i + 1) * 128],
                                      xa[0:64, MAR + o + dx:MAR + o + dx + CN],
                                      None))
                    for i, dx in enumerate((-1, 0, 1)):
                        units.append((wpair[:, i * 128:(i + 1) * 128],
                                      xa[0:128, MAR + o - PW + dx:
                                         MAR + o - PW + dx + CN], None))
                    # fp8 DoubleRow corr: plane0/plane1 pair taps (-1,dx)
                    # with (+1,dx) (delta 2*PW) and (0,-1) with (0,+1)
                    for i, dx in enumerate((-1, 0, 1)):
                        b0 = MAR + o - PW + dx
                        units.append((wc66_s[:, 0:2, i * 128:(i + 1) * 128],
                                      x6[:, 0:2, b0:b0 + CN], DR))
                    units.append((wc2_s[:, 0:2, :],
                                  x2[:, 0:2, 2 + o - 1:2 + o - 1 + CN], DR))
                    # tap (0,0) as DoubleRow with a zeroed second plane
                    units.append((wc0_s[:, 0:2, :],
                                  x6[:, 0:2, MAR + o:MAR + o + CN], DR))
                    for k, (w_ap, x_ap, pm) in enumerate(units):
                        nc.tensor.matmul(ps[:], w_ap, x_ap,
                                         start=(k == 0),
                                         stop=(k == len(units) - 1),
                                         perf_mode=pm)
                    nc.scalar.activation(
                        y[:, o:o + CN], ps[:], AF.Identity,
                        bias=bias_t[:, 0:1], scale=1.0 / 65536.0,
                        accum_out=s_st[s][:, c * T + t:c * T + t + 1])
                # stats: -junk sum, max over real cols, total
                yj = y[:, MAR:MAR + 32 * PW].rearrange(
                    "p (r c) -> p r c", c=PW)
                nc.vector.reduce_sum(s_st[s][:, 3 * T + t:3 * T + t + 1],
                                     yj[:, :, 0:1], axis=AX.XY, negate=True)
                ym = y[:, OUT0:OUT0 + 32 * PW].rearrange(
                    "p (r c) -> p r c", c=PW)
                nc.vector.reduce_max(s_st[s][:, 5 * T + t:5 * T + t + 1],
                                     ym[:, :, 0:W], axis=AX.XY)
                sv = s_st[s].rearrange("p (k t) -> p k t", t=T)
                nc.vector.reduce_sum(sv[:, 4:5, t:t + 1], sv[:, 0:4, t:t + 1],
                                     axis=AX.XY)

            def attention(s):
                S = s_st[s]
                psT1 = PP.tile([T, 128], F32, tag="pa0", name="psT1")
                nc.tensor.transpose(psT1[:], S[:, 4 * T:5 * T], ident[:])
                psT2 = PP.tile([T, 128], F32, tag="pa1", name="psT2")
                nc.tensor.transpose(psT2[:], S[:, 5 * T:6 * T], ident[:])
                tmp = P2.tile([T, 1], F32, tag="att_tmp", name="att_tmp")
                nc.vector.reduce_sum(tmp[:], psT1[:], axis=AX.X)
                att_in = P2.tile([T, 2], F32, tag="att_in", name="att_in")
                nc.vector.tensor_scalar_mul(att_in[:, 0:1], tmp[:],
                                            1.0 / (CH * HW))
                nc.vector.reduce_max(att_in[:, 1:2], psT2[:], axis=AX.X)
                ps5 = PP.tile([5, 2], F32, tag="pa0", name="ps5")
                nc.tensor.matmul(ps5[:], w1t_s[:], att_in[:], start=True,
                                 stop=True)
                h5 = P2.tile([5, 2], F32, tag="h5", name="h5")
                nc.vector.tensor_scalar_max(h5[:], ps5[:], 0.0)
                ps20 = PP.tile([T, 2], F32, tag="pa1", name="ps20")
                nc.tensor.matmul(ps20[:], w2t_s[:], h5[:], start=True, stop=True)
                a20 = P2.tile([T, 2], F32, tag="a20", name="a20")
                nc.vector.tensor_scalar_add(a20[:], ps20[:], 0.0)
                attp = P2.tile([T, 1], F32, tag="attp", name="attp")
                nc.vector.tensor_tensor(attp[:], a20[:, 0:1], a20[:, 1:2],
                                        op=OP.add)
                # sigmoid via exp + reciprocal (tighter than the Sigmoid table)
                expz = P2.tile([T, 1], F32, tag="expz", name="expz")
                nc.scalar.activation(expz[:], attp[:], AF.Exp, scale=-1.0)
                att1 = P2.tile([T, 1], F32, tag="att1", name="att1")
                nc.vector.tensor_scalar_add(att1[:], expz[:], 1.0)
                att = P2.tile([T, 1], F32, tag="att", name="att")
                nc.vector.reciprocal(att[:], att1[:])
                psT3 = PP.tile([1, T], F32, tag="pa0", name="psT3")
                nc.tensor.transpose(psT3[:], att[:, 0:1], ident[0:T, 0:T])
                atts = P2.tile([1, T + 1], F32, tag="atts", name="atts")
                nc.vector.tensor_scalar_add(atts[0:1, 1:T + 1], psT3[:], 0.0)
                nc.vector.tensor_scalar_add(atts[0:1, 0:1], psT3[0:1, 0:1],
                                            0.0)
                rec = P2.tile([1, T], F32, tag="rec", name="rec")
                nc.vector.reciprocal(rec[:], atts[0:1, 1:T + 1])
                rhs3 = P2.tile([1, 4 * T], F32, tag="rhs3", name="rhs3")
                nc.vector.scalar_tensor_tensor(
                    rhs3[0:1, 0:T], atts[0:1, 0:T], ALPHA, rec[:],
                    op0=OP.mult, op1=OP.mult)
                nc.vector.tensor_scalar_mul(rhs3[0:1, T:2 * T], rec[:], VTH)
                nc.vector.tensor_scalar_mul(rhs3[0:1, 2 * T:3 * T], rec[:],
                                            -VTH)
                nc.vector.tensor_scalar_mul(rhs3[0:1, 3 * T:4 * T], rec[:],
                                            -VTH * 1e8)
                ps_bc = PP.tile([128, 4 * T], F32, tag="pa1", name="ps_bc")
                nc.tensor.matmul(ps_bc[:], ones_t[:], rhs3[:], start=True,
                                 stop=True)
                nc.vector.tensor_scalar_add(bc[s][:], ps_bc[:], 0.0)

            def scan_step(s, t, vg, sp):
                f = s * T + t
                g = gs[s]
                if t == 0:
                    nc.vector.memset(g[:], 0.0)
                y = ys[f % NY]
                yv = y[:, OUT0:OUT0 + 32 * PW].rearrange(
                    "p (r c) -> p r c", c=PW)
                v = P2.tile([128, HW], F32, tag="v", name="v")
                m = (P2.tile([128, HW], F32, tag="m", name="m")
                     if any(e == "p" for e, _, _ in vg) else None)
                so = P3.tile([128, HW], FP8, tag="so", name="so")
                vv = v.rearrange("p (r c) -> p r c", c=W)
                gv = g.rearrange("p (r c) -> p r c", c=W)
                cb = bc[s][:, t:t + 1]
                tn = min(t + 1, T - 1)
                cbn = bc[s][:, tn:tn + 1]
                thr = bc[s][:, T + t:T + t + 1]
                nthr = bc[s][:, 2 * T + t:2 * T + t + 1]
                nthr8 = bc[s][:, 3 * T + t:3 * T + t + 1]
                for eng, r0, r1 in vg:
                    R = slice(r0 // W, r1 // W)
                    if eng == "v":
                        nc.vector.scalar_tensor_tensor(
                            vv[:, R, :], gv[:, R, :], cb, yv[:, R, 0:W],
                            op0=OP.mult, op1=OP.add)
                        nc.vector.scalar_tensor_tensor(
                            g[:, r0:r1], v[:, r0:r1], thr, v[:, r0:r1],
                            op0=OP.is_lt, op1=OP.mult)
                    else:
                        # Pool rows keep g pre-multiplied by c_{t+1}:
                        # v = g + y; m = (v<thr)*c_next; g = m*v
                        nc.gpsimd.tensor_tensor(
                            vv[:, R, :], gv[:, R, :], yv[:, R, 0:W],
                            op=OP.add)
                        nc.gpsimd.tensor_scalar(
                            m[:, r0:r1], v[:, r0:r1], thr, cbn,
                            op0=OP.is_lt, op1=OP.mult)
                        nc.gpsimd.tensor_tensor(
                            g[:, r0:r1], m[:, r0:r1], v[:, r0:r1],
                            op=OP.mult)
                for eng, r0, r1 in sp:
                    if eng == "sig":
                        # saturated sigmoid: 1e8*(v - thr) is past the f32
                        # sigmoid saturation point except ~1e-7 from thr
                        nc.scalar.activation(so[:, r0:r1], v[:, r0:r1],
                                             AF.Sigmoid, bias=nthr8,
                                             scale=1e8)
                    elif eng == "pm":
                        # spike from m (= (v<thr)*c_next): exactly 0 iff spike
                        nc.gpsimd.tensor_scalar(
                            so[:, r0:r1], m[:, r0:r1], 0.0, None,
                            op0=OP.is_equal)
                    else:
                        nc.vector.tensor_scalar(
                            so[:, r0:r1], v[:, r0:r1], thr, None,
                            op0=OP.is_ge)
                nc.sync.dma_start(spk[s, t], so[:])

            OVERLAP_VG = [("v", 0, 896), ("p", 896, HW)]
            OVERLAP_SP = [("sig", 0, HW)]
            TAIL_VG = [("v", 0, 368), ("v", 368, 736), ("p", 736, HW)]
            TAIL_SP = [("sig", 0, HW)]

            conv_frame(0, 0, skip_dma=True)
            for t in range(1, T):
                conv_frame(0, t)
            for t in range(4):
                conv_frame(1, t)
            # att(0) after 4 conv(1) frames: its PE ops sit behind queued
            # conv matmuls while the DVE/ACT chain resolves
            attention(0)
            # input prefetch 2 frames ahead: the spk DMA inside scan_step
            # waits on the scan result and blocks the SP queue behind it
            x_dma(1, 4)
            x_dma(1, 5)
            for t in range(T - 4):
                scan_step(0, t, OVERLAP_VG, OVERLAP_SP)
                conv_frame(1, t + 4, skip_dma=True)
                if t + 6 < T:
                    x_dma(1, t + 6)
            attention(1)
            for t in range(T - 4, T):
                scan_step(0, t, OVERLAP_VG, OVERLAP_SP)
            for t in range(T):
                scan_step(1, t, TAIL_VG, TAIL_SP)

    nc.compile()
    return nc


def _trunc13(a):
    # f32r = round-to-nearest, 11 explicit mantissa bits (HW-verified via
    # DMA roundtrip). Split values must be 11-bit so the hardware re-round
    # is a no-op and x_hi + x_lo == x exactly.
    u = np.ascontiguousarray(a, np.float32).view(np.uint32)
    r = (u + np.uint32(0x800)) & np.uint32(0xFFFFF000)
    return r.view(np.float32)


def _pad_frames(x):
    """[.., 64, 32, 32] -> [.., 64, XCOL] host-packed shared-halo frames."""
    lead = x.shape[:-2]
    padded = np.zeros(lead + (34, PW), np.float32)
    padded[..., 1:33, 1:33] = x
    out = np.zeros(lead + (XCOL,), np.float32)
    out[..., :34 * PW] = padded.reshape(lead + (34 * PW,))
    return out


E4M3 = ml_dtypes.float8_e4m3fn


def _fp8(a):
    return np.asarray(a, np.float32).astype(E4M3)


def _prep_host_inputs(conv_w, conv_b, mlp_w1, mlp_w2):
    wT = np.ascontiguousarray(np.transpose(conv_w, (1, 0, 2, 3)))  # [64,128,3,3]
    hi = {}
    c8 = {}
    for dy, dx in TAPS:
        blk = np.ascontiguousarray(wT[:, :, dy + 1, dx + 1])
        h = _trunc13(blk)
        lo = (blk - h).astype(np.float32)
        hi[(dy, dx)] = h
        # fp8 corr weights: [w_lo*2^16 ; w_hi*2^4] (psum scale 2^16 with
        # x_lo prescaled by 2^12 on the data side)
        c8[(dy, dx)] = np.concatenate(
            [_fp8(lo * 65536.0), _fp8(h * 16.0)], axis=0)          # [128,128]
    # T1 weights prescaled by 2^16 (exact) to share the corr psum scale
    wpair = np.concatenate(
        [np.concatenate([hi[(-1, dx)], hi[(1, dx)]], axis=0)
         for dx in (-1, 0, 1)], axis=1) * 65536.0                  # [128, 384]
    wsing = np.concatenate(
        [hi[(0, dx)] for dx in (-1, 0, 1)], axis=1) * 65536.0
    wc66 = np.stack(
        [np.concatenate([c8[(-1, dx)] for dx in (-1, 0, 1)], axis=1),
         np.concatenate([c8[(1, dx)] for dx in (-1, 0, 1)], axis=1)],
        axis=1)                                                    # [128,2,384]
    wc2 = np.stack([c8[(0, -1)], c8[(0, 1)]], axis=1)              # [128,2,128]
    return {
        "wpair": np.ascontiguousarray(wpair, np.float32),
        "wsing": np.ascontiguousarray(wsing, np.float32),
        "wc66": np.ascontiguousarray(wc66),
        "wc2": np.ascontiguousarray(wc2),
        "wc0": np.ascontiguousarray(
            np.stack([c8[(0, 0)], np.zeros_like(c8[(0, 0)])], axis=1)),
        "bias": np.ascontiguousarray(conv_b.reshape(128, 1), np.float32),
        "w1t": np.ascontiguousarray(mlp_w1.T).astype(np.float32),
        "w2t": np.ascontiguousarray(mlp_w2.T).astype(np.float32),
        "ident": np.eye(128, dtype=np.float32),
    }


_CACHED = {}


def make_in_maps(data, conv_w, conv_b, mlp_w1, mlp_w2):
    data = np.ascontiguousarray(data, np.float32)
    common = _prep_host_inputs(np.asarray(conv_w, np.float32),
                               np.asarray(conv_b, np.float32),
                               np.asarray(mlp_w1, np.float32),
                               np.asarray(mlp_w2, np.float32))
    in_maps = []
    for c in range(N_CORES):
        m = dict(common)
        shard = _pad_frames(data[c * BPC:(c + 1) * BPC])
        h = _trunc13(shard)
        m["xhi"] = h
        # fp8 corr data: [fp8(x_hi) ; fp8(x_lo*2^12)] in two shifted planes
        c8 = np.concatenate(
            [_fp8(h), _fp8((shard - h) * 4096.0)], axis=2)  # [BPC,T,128,XCOL]
        x66 = np.zeros((BPC, T, 128, 2, MAR + XCOL), E4M3)
        x66[:, :, :, 0, MAR:MAR + XCOL] = c8
        x66[:, :, :, 1, 0:XCOL] = c8
        m["xc66"] = x66
        x2 = np.zeros((BPC, T, 128, 2, 2 + XCOL), E4M3)
        x2[:, :, :, 0, 2:2 + XCOL] = c8
        x2[:, :, :, 1, 0:XCOL] = c8
        m["xc2"] = x2
        in_maps.append(m)
    return in_maps


def kernel(data, conv_w, conv_b, mlp_w1, mlp_w2):
    if "prog" not in _CACHED:
        _CACHED["prog"] = _build_program()
    nc = _CACHED["prog"]
    in_maps = make_in_maps(data, conv_w, conv_b, mlp_w1, mlp_w2)
    res = run_bass_kernel_spmd(nc, in_maps, list(range(N_CORES)))
    out = np.concatenate(
        [np.asarray(res.results[c]["spk"]).astype(np.float32)
         for c in range(N_CORES)], axis=0)
    return out.reshape(B, T, CH, H, W)


# revision 18
# speedup vs baseline: 1.0030x; 1.0030x over previous
"""Trainium2 Bass kernel for nn_ConvAttLIF (conv3x3 + temporal attention + LIF scan).

Sharding: data-parallel over batch B=16 across 8 NeuronCores (2 samples/core).

Layout: frames host-packed with shared row halos (33-wide rows: the right
halo of row r is the left halo of row r+1, both zero), so a frame is 1124
contiguous cols and the conv output span is 1056 cols = 3 psum chunks of 352.

Conv: per chunk, 15 f32r matmuls accumulate one psum bank:
  - 3 "pair" units (K=128): taps (-1,dx) and (+1,dx) fused by storing a
    second frame copy shifted 2 rows (66 cols) in partitions 64-127.
  - 3 "single" units (K=64): taps (0,dx) on partitions 0-63.
  - 9 "corr" units (K=128): [x_hi; x_lo] . [w_lo; w_hi] per tap, restoring
    ~fp32 accuracy from the 12-bit f32r operands (x_hi = trunc13(x)).
Chunks are processed in rotating order (frame f starts at chunk f%3) so each
frame's first psum bank was drained one chunk-stream earlier - no PE stall.

LIF scan: attention folded in via v_t = u_t/att_t, so each step is
v = g*c_t + y (STT), g = (v < thr_t)*v (STT, same engine - no cross-engine
hop in the serial chain), spike = (v >= thr_t) off-chain. The sample-1 tail
(no conv left to overlap) splits rows across DVE/Pool/ACT.

kernel(**inputs) takes the FULL unsharded inputs, returns the FULL output.
"""
import sys

sys.path.insert(0, "/opt/trn_rl_repo")

import numpy as np
import ml_dtypes
import concourse.bass as bass
import concourse.bacc as bacc
import concourse.tile as tile
import concourse.mybir as mybir
from concourse.bass_utils import run_bass_kernel_spmd

F32 = mybir.dt.float32
F32R = mybir.dt.float32r
FP8 = mybir.dt.float8e4
BF16 = mybir.dt.bfloat16
DR = mybir.MatmulPerfMode.DoubleRow
AF = mybir.ActivationFunctionType
OP = mybir.AluOpType
AX = mybir.AxisListType

B, T, CIN, H, W = 16, 20, 64, 32, 32
CH = 128
N_CORES = 8
BPC = B // N_CORES
ALPHA, VTH = 0.3, 0.6
HW = H * W                     # 1024
PW = W + 1                     # 33: row stride (shared halo col)
XCOL = 34 * PW + 2             # 1124 packed frame cols (+2 guard)
MAR = 2 * PW                   # 66: left margin in XA for the shifted copy
CN = 352                       # psum chunk cols (3 x 352 = 1056 out span)
OUT0 = PW + 1                  # 34: first out position in frame coords
NY = 25                        # y-tile ring size
TAPS = [(dy, dx) for dy in (-1, 0, 1) for dx in (-1, 0, 1)]


def _build_program():
    nc = bacc.Bacc("TRN2", target_bir_lowering=False, debug=False,
                   num_devices=N_CORES)

    xhi_d = nc.dram_tensor("xhi", [BPC, T, CIN, XCOL], F32,
                           kind="ExternalInput").ap()
    xc66_d = nc.dram_tensor("xc66", [BPC, T, 128, 2, MAR + XCOL], FP8,
                            kind="ExternalInput").ap()
    xc2_d = nc.dram_tensor("xc2", [BPC, T, 128, 2, 2 + XCOL], FP8,
                           kind="ExternalInput").ap()
    wpair_d = nc.dram_tensor("wpair", [128, 3 * 128], F32,
                             kind="ExternalInput").ap()
    wsing_d = nc.dram_tensor("wsing", [64, 3 * 128], F32,
                             kind="ExternalInput").ap()
    wc66_d = nc.dram_tensor("wc66", [128, 2, 3 * 128], FP8,
                            kind="ExternalInput").ap()
    wc2_d = nc.dram_tensor("wc2", [128, 2, 128], FP8,
                           kind="ExternalInput").ap()
    wc0_d = nc.dram_tensor("wc0", [128, 2, 128], FP8,
                           kind="ExternalInput").ap()
    bias_d = nc.dram_tensor("bias", [128, 1], F32, kind="ExternalInput").ap()
    w1t_d = nc.dram_tensor("w1t", [T, 5], F32, kind="ExternalInput").ap()
    w2t_d = nc.dram_tensor("w2t", [5, T], F32, kind="ExternalInput").ap()
    ident_d = nc.dram_tensor("ident", [128, 128], F32, kind="ExternalInput").ap()
    spk = nc.dram_tensor("spk", [BPC, T, CH, HW], FP8,
                         kind="ExternalOutput").ap()

    with tile.TileContext(nc) as tc:
        with tc.tile_pool(name="sb", bufs=1) as P1, \
             tc.tile_pool(name="scr", bufs=2) as P2, \
             tc.tile_pool(name="so", bufs=3) as P3, \
             tc.tile_pool(name="ps", bufs=1, space="PSUM") as PP:

            # ---- persistent tiles ----
            xas = [P1.tile([128, MAR + XCOL], F32R, tag=f"xa{i}", name=f"xa{i}")
                   for i in range(4)]
            xc66s = [P1.tile([128, 2, MAR + XCOL], FP8, tag=f"x6{i}",
                             name=f"x6{i}") for i in range(4)]
            xc2s = [P1.tile([128, 2, 2 + XCOL], FP8, tag=f"x2{i}",
                            name=f"x2{i}") for i in range(4)]

            def x_dma(s, t):
                f = s * T + t
                src = xhi_d[s, t].bitcast(F32R)
                nc.sync.dma_start(xas[f % 4][0:64, MAR:MAR + XCOL], src)
                nc.sync.dma_start(xas[f % 4][64:128, 0:XCOL], src)
                nc.sync.dma_start(xc66s[f % 4][:], xc66_d[s, t])
                nc.sync.dma_start(xc2s[f % 4][:], xc2_d[s, t])

            # startup order: frame-0 XA halves, pair/single weights (first
            # units of the first chunk), then the corr inputs
            f0src = xhi_d[0, 0].bitcast(F32R)
            nc.sync.dma_start(xas[0][0:64, MAR:MAR + XCOL], f0src)
            wsing = P1.tile([64, 3 * 128], F32R, tag="wsing", name="wsing")
            nc.sync.dma_start(wsing[:], wsing_d[:].bitcast(F32R))
            nc.sync.dma_start(xas[0][64:128, 0:XCOL], f0src)
            wpair = P1.tile([128, 3 * 128], F32R, tag="wpair", name="wpair")
            nc.sync.dma_start(wpair[:], wpair_d[:].bitcast(F32R))
            nc.sync.dma_start(xc66s[0][:], xc66_d[0, 0])
            nc.sync.dma_start(xc2s[0][:], xc2_d[0, 0])
            bias_t = P1.tile([128, 1], F32, tag="bias", name="bias")
            nc.sync.dma_start(bias_t[:], bias_d[:])

            wc66_s = P1.tile([128, 2, 3 * 128], FP8, tag="wc66", name="wc66")
            nc.sync.dma_start(wc66_s[:], wc66_d[:])
            wc2_s = P1.tile([128, 2, 128], FP8, tag="wc2", name="wc2")
            nc.sync.dma_start(wc2_s[:], wc2_d[:])
            wc0_s = P1.tile([128, 2, 128], FP8, tag="wc0", name="wc0")
            nc.sync.dma_start(wc0_s[:], wc0_d[:])
            w1t_s = P1.tile([T, 5], F32, tag="w1t", name="w1t")
            nc.sync.dma_start(w1t_s[:], w1t_d[:])
            w2t_s = P1.tile([5, T], F32, tag="w2t", name="w2t")
            nc.sync.dma_start(w2t_s[:], w2t_d[:])
            ident = P1.tile([128, 128], F32, tag="ident", name="ident")
            nc.sync.dma_start(ident[:], ident_d[:])
            ones_t = P1.tile([1, 128], F32, tag="ones", name="ones")
            nc.vector.memset(ones_t[:], 1.0)

            ys = [P1.tile([128, XCOL], F32, tag=f"y{i}", name=f"y{i}")
                  for i in range(NY)]
            gs = [P1.tile([128, HW], F32, tag=f"g{s}", name=f"g{s}")
                  for s in range(BPC)]
            # stats rows: 0-2 chunk sums, 3 -junk, 4 total, 5 max
            s_st = [P1.tile([128, 6 * T], F32, tag=f"S{s}", name=f"S{s}")
                    for s in range(BPC)]
            bc = [P1.tile([128, 4 * T], F32, tag=f"bc{s}", name=f"bc{s}")
                  for s in range(BPC)]

            engines = {"v": nc.vector, "p": nc.gpsimd}

            def conv_frame(s, t, skip_dma=False):
                f = s * T + t
                if not skip_dma:
                    x_dma(s, t)
                xa, x6, x2 = xas[f % 4], xc66s[f % 4], xc2s[f % 4]
                y = ys[f % NY]
                for ci in range(3):
                    c = (f + ci) % 3
                    o = OUT0 + CN * c
                    ps = PP.tile([128, CN], F32, tag=f"p{c}{f % 2}",
                                 name=f"p{c}{f % 2}")
                    units = []
                    for i, dx in enumerate((-1, 0, 1)):
                        units.append((wsing[:, i * 128:(i + 1) * 128],
                                      xa[0:64, MAR + o + dx:MAR + o + dx + CN],
                                      None))
                    for i, dx in enumerate((-1, 0, 1)):
                        units.append((wpair[:, i * 128:(i + 1) * 128],
                                      xa[0:128, MAR + o - PW + dx:
                                         MAR + o - PW + dx + CN], None))
                    # fp8 DoubleRow corr: plane0/plane1 pair taps (-1,dx)
                    # with (+1,dx) (delta 2*PW) and (0,-1) with (0,+1)
                    for i, dx in enumerate((-1, 0, 1)):
                        b0 = MAR + o - PW + dx
                        units.append((wc66_s[:, 0:2, i * 128:(i + 1) * 128],
                                      x6[:, 0:2, b0:b0 + CN], DR))
                    units.append((wc2_s[:, 0:2, :],
                                  x2[:, 0:2, 2 + o - 1:2 + o - 1 + CN], DR))
                    # tap (0,0) as DoubleRow with a zeroed second plane
                    units.append((wc0_s[:, 0:2, :],
                                  x6[:, 0:2, MAR + o:MAR + o + CN], DR))
                    for k, (w_ap, x_ap, pm) in enumerate(units):
                        nc.tensor.matmul(ps[:], w_ap, x_ap,
                                         start=(k == 0),
                                         stop=(k == len(units) - 1),
                                         perf_mode=pm)
                    nc.scalar.activation(
                        y[:, o:o + CN], ps[:], AF.Identity,
                        bias=bias_t[:, 0:1], scale=1.0 / 65536.0,
                        accum_out=s_st[s][:, c * T + t:c * T + t + 1])
                # stats: -junk sum, max over real cols, total
                yj = y[:, MAR:MAR + 32 * PW].rearrange(
                    "p (r c) -> p r c", c=PW)
                nc.vector.reduce_sum(s_st[s][:, 3 * T + t:3 * T + t + 1],
                                     yj[:, :, 0:1], axis=AX.XY, negate=True)
                ym = y[:, OUT0:OUT0 + 32 * PW].rearrange(
                    "p (r c) -> p r c", c=PW)
                nc.vector.reduce_max(s_st[s][:, 5 * T + t:5 * T + t + 1],
                                     ym[:, :, 0:W], axis=AX.XY)
                sv = s_st[s].rearrange("p (k t) -> p k t", t=T)
                nc.vector.reduce_sum(sv[:, 4:5, t:t + 1], sv[:, 0:4, t:t + 1],
                                     axis=AX.XY)

            def attention(s):
                S = s_st[s]
                psT1 = PP.tile([T, 128], F32, tag="pa0", name="psT1")
                nc.tensor.transpose(psT1[:], S[:, 4 * T:5 * T], ident[:])
                psT2 = PP.tile([T, 128], F32, tag="pa1", name="psT2")
                nc.tensor.transpose(psT2[:], S[:, 5 * T:6 * T], ident[:])
                tmp = P2.tile([T, 1], F32, tag="att_tmp", name="att_tmp")
                nc.vector.reduce_sum(tmp[:], psT1[:], axis=AX.X)
                att_in = P2.tile([T, 2], F32, tag="att_in", name="att_in")
                nc.vector.tensor_scalar_mul(att_in[:, 0:1], tmp[:],
                                            1.0 / (CH * HW))
                nc.vector.reduce_max(att_in[:, 1:2], psT2[:], axis=AX.X)
                ps5 = PP.tile([5, 2], F32, tag="pa0", name="ps5")
                nc.tensor.matmul(ps5[:], w1t_s[:], att_in[:], start=True,
                                 stop=True)
                h5 = P2.tile([5, 2], F32, tag="h5", name="h5")
                nc.vector.tensor_scalar_max(h5[:], ps5[:], 0.0)
                ps20 = PP.tile([T, 2], F32, tag="pa1", name="ps20")
                nc.tensor.matmul(ps20[:], w2t_s[:], h5[:], start=True, stop=True)
                a20 = P2.tile([T, 2], F32, tag="a20", name="a20")
                nc.vector.tensor_scalar_add(a20[:], ps20[:], 0.0)
                attp = P2.tile([T, 1], F32, tag="attp", name="attp")
                nc.vector.tensor_tensor(attp[:], a20[:, 0:1], a20[:, 1:2],
                                        op=OP.add)
                # sigmoid via exp + reciprocal (tighter than the Sigmoid table)
                expz = P2.tile([T, 1], F32, tag="expz", name="expz")
                nc.scalar.activation(expz[:], attp[:], AF.Exp, scale=-1.0)
                att1 = P2.tile([T, 1], F32, tag="att1", name="att1")
                nc.vector.tensor_scalar_add(att1[:], expz[:], 1.0)
                att = P2.tile([T, 1], F32, tag="att", name="att")
                nc.vector.reciprocal(att[:], att1[:])
                psT3 = PP.tile([1, T], F32, tag="pa0", name="psT3")
                nc.tensor.transpose(psT3[:], att[:, 0:1], ident[0:T, 0:T])
                atts = P2.tile([1, T + 1], F32, tag="atts", name="atts")
                nc.vector.tensor_scalar_add(atts[0:1, 1:T + 1], psT3[:], 0.0)
                nc.vector.tensor_scalar_add(atts[0:1, 0:1], psT3[0:1, 0:1],
                                            0.0)
                rec = P2.tile([1, T], F32, tag="rec", name="rec")
                nc.vector.reciprocal(rec[:], atts[0:1, 1:T + 1])
                rhs3 = P2.tile([1, 4 * T], F32, tag="rhs3", name="rhs3")
                nc.vector.scalar_tensor_tensor(
                    rhs3[0:1, 0:T], atts[0:1, 0:T], ALPHA, rec[:],
                    op0=OP.mult, op1=OP.mult)
                nc.vector.tensor_scalar_mul(rhs3[0:1, T:2 * T], rec[:], VTH)
                nc.vector.tensor_scalar_mul(rhs3[0:1, 2 * T:3 * T], rec[:],
                                            -VTH)
                nc.vector.tensor_scalar_mul(rhs3[0:1, 3 * T:4 * T], rec[:],
                                            -VTH * 1e8)
                ps_bc = PP.tile([128, 4 * T], F32, tag="pa1", name="ps_bc")
                nc.tensor.matmul(ps_bc[:], ones_t[:], rhs3[:], start=True,
                                 stop=True)
                nc.vector.tensor_scalar_add(bc[s][:], ps_bc[:], 0.0)

            def scan_step(s, t, vg, sp):
                f = s * T + t
                g = gs[s]
                if t == 0:
                    nc.vector.memset(g[:], 0.0)
                y = ys[f % NY]
                yv = y[:, OUT0:OUT0 + 32 * PW].rearrange(
                    "p (r c) -> p r c", c=PW)
                v = P2.tile([128, HW], F32, tag="v", name="v")
                m = (P2.tile([128, HW], F32, tag="m", name="m")
                     if any(e == "p" for e, _, _ in vg) else None)
                so = P3.tile([128, HW], FP8, tag="so", name="so")
                vv = v.rearrange("p (r c) -> p r c", c=W)
                gv = g.rearrange("p (r c) -> p r c", c=W)
                cb = bc[s][:, t:t + 1]
                tn = min(t + 1, T - 1)
                cbn = bc[s][:, tn:tn + 1]
                thr = bc[s][:, T + t:T + t + 1]
                nthr = bc[s][:, 2 * T + t:2 * T + t + 1]
                nthr8 = bc[s][:, 3 * T + t:3 * T + t + 1]
                for eng, r0, r1 in vg:
                    R = slice(r0 // W, r1 // W)
                    if eng == "v":
                        nc.vector.scalar_tensor_tensor(
                            vv[:, R, :], gv[:, R, :], cb, yv[:, R, 0:W],
                            op0=OP.mult, op1=OP.add)
                        nc.vector.scalar_tensor_tensor(
                            g[:, r0:r1], v[:, r0:r1], thr, v[:, r0:r1],
                            op0=OP.is_lt, op1=OP.mult)
                    else:
                        # Pool rows keep g pre-multiplied by c_{t+1}:
                        # v = g + y; m = (v<thr)*c_next; g = m*v
                        nc.gpsimd.tensor_tensor(
                            vv[:, R, :], gv[:, R, :], yv[:, R, 0:W],
                            op=OP.add)
                        nc.gpsimd.tensor_scalar(
                            m[:, r0:r1], v[:, r0:r1], thr, cbn,
                            op0=OP.is_lt, op1=OP.mult)
                        nc.gpsimd.tensor_tensor(
                            g[:, r0:r1], m[:, r0:r1], v[:, r0:r1],
                            op=OP.mult)
                for eng, r0, r1 in sp:
                    if eng == "sig":
                        # saturated sigmoid: 1e8*(v - thr) is past the f32
                        # sigmoid saturation point except ~1e-7 from thr
                        nc.scalar.activation(so[:, r0:r1], v[:, r0:r1],
                                             AF.Sigmoid, bias=nthr8,
                                             scale=1e8)
                    elif eng == "pm":
                        # spike from m (= (v<thr)*c_next): exactly 0 iff spike
                        nc.gpsimd.tensor_scalar(
                            so[:, r0:r1], m[:, r0:r1], 0.0, None,
                            op0=OP.is_equal)
                    else:
                        nc.vector.tensor_scalar(
                            so[:, r0:r1], v[:, r0:r1], thr, None,
                            op0=OP.is_ge)
                nc.sync.dma_start(spk[s, t], so[:])

            OVERLAP_VG = [("v", 0, 896), ("p", 896, HW)]
            OVERLAP_SP = [("sig", 0, HW)]
            TAIL_VG = [("v", 0, 352), ("v", 352, 736), ("p", 736, HW)]
            TAIL_SP = [("sig", 0, HW)]

            conv_frame(0, 0, skip_dma=True)
            for t in range(1, T):
                conv_frame(0, t)
            for t in range(4):
                conv_frame(1, t)
            # att(0) after 4 conv(1) frames: its PE ops sit behind queued
            # conv matmuls while the DVE/ACT chain resolves
            attention(0)
            # input prefetch 2 frames ahead: the spk DMA inside scan_step
            # waits on the scan result and blocks the SP queue behind it
            x_dma(1, 4)
            x_dma(1, 5)
            for t in range(T - 4):
                scan_step(0, t, OVERLAP_VG, OVERLAP_SP)
                conv_frame(1, t + 4, skip_dma=True)
                if t + 6 < T:
                    x_dma(1, t + 6)
            attention(1)
            for t in range(T - 4, T):
                scan_step(0, t, OVERLAP_VG, OVERLAP_SP)
            for t in range(T):
                scan_step(1, t, TAIL_VG, TAIL_SP)

    nc.compile()
    return nc


def _trunc13(a):
    # f32r = round-to-nearest, 11 explicit mantissa bits (HW-verified via
    # DMA roundtrip). Split values must be 11-bit so the hardware re-round
    # is a no-op and x_hi + x_lo == x exactly.
    u = np.ascontiguousarray(a, np.float32).view(np.uint32)
    r = (u + np.uint32(0x800)) & np.uint32(0xFFFFF000)
    return r.view(np.float32)


def _pad_frames(x):
    """[.., 64, 32, 32] -> [.., 64, XCOL] host-packed shared-halo frames."""
    lead = x.shape[:-2]
    padded = np.zeros(lead + (34, PW), np.float32)
    padded[..., 1:33, 1:33] = x
    out = np.zeros(lead + (XCOL,), np.float32)
    out[..., :34 * PW] = padded.reshape(lead + (34 * PW,))
    return out


E4M3 = ml_dtypes.float8_e4m3fn


def _fp8(a):
    return np.asarray(a, np.float32).astype(E4M3)


def _prep_host_inputs(conv_w, conv_b, mlp_w1, mlp_w2):
    wT = np.ascontiguousarray(np.transpose(conv_w, (1, 0, 2, 3)))  # [64,128,3,3]
    hi = {}
    c8 = {}
    for dy, dx in TAPS:
        blk = np.ascontiguousarray(wT[:, :, dy + 1, dx + 1])
        h = _trunc13(blk)
        lo = (blk - h).astype(np.float32)
        hi[(dy, dx)] = h
        # fp8 corr weights: [w_lo*2^16 ; w_hi*2^4] (psum scale 2^16 with
        # x_lo prescaled by 2^12 on the data side)
        c8[(dy, dx)] = np.concatenate(
            [_fp8(lo * 65536.0), _fp8(h * 16.0)], axis=0)          # [128,128]
    # T1 weights prescaled by 2^16 (exact) to share the corr psum scale
    wpair = np.concatenate(
        [np.concatenate([hi[(-1, dx)], hi[(1, dx)]], axis=0)
         for dx in (-1, 0, 1)], axis=1) * 65536.0                  # [128, 384]
    wsing = np.concatenate(
        [hi[(0, dx)] for dx in (-1, 0, 1)], axis=1) * 65536.0
    wc66 = np.stack(
        [np.concatenate([c8[(-1, dx)] for dx in (-1, 0, 1)], axis=1),
         np.concatenate([c8[(1, dx)] for dx in (-1, 0, 1)], axis=1)],
        axis=1)                                                    # [128,2,384]
    wc2 = np.stack([c8[(0, -1)], c8[(0, 1)]], axis=1)              # [128,2,128]
    return {
        "wpair": np.ascontiguousarray(wpair, np.float32),
        "wsing": np.ascontiguousarray(wsing, np.float32),
        "wc66": np.ascontiguousarray(wc66),
        "wc2": np.ascontiguousarray(wc2),
        "wc0": np.ascontiguousarray(
            np.stack([c8[(0, 0)], np.zeros_like(c8[(0, 0)])], axis=1)),
        "bias": np.ascontiguousarray(conv_b.reshape(128, 1), np.float32),
        "w1t": np.ascontiguousarray(mlp_w1.T).astype(np.float32),
        "w2t": np.ascontiguousarray(mlp_w2.T).astype(np.float32),
        "ident": np.eye(128, dtype=np.float32),
    }


_CACHED = {}


def make_in_maps(data, conv_w, conv_b, mlp_w1, mlp_w2):
    data = np.ascontiguousarray(data, np.float32)
    common = _prep_host_inputs(np.asarray(conv_w, np.float32),
                               np.asarray(conv_b, np.float32),
                               np.asarray(mlp_w1, np.float32),
                               np.asarray(mlp_w2, np.float32))
    in_maps = []
    for c in range(N_CORES):
        m = dict(common)
        shard = _pad_frames(data[c * BPC:(c + 1) * BPC])
        h = _trunc13(shard)
        m["xhi"] = h
        # fp8 corr data: [fp8(x_hi) ; fp8(x_lo*2^12)] in two shifted planes
        c8 = np.concatenate(
            [_fp8(h), _fp8((shard - h) * 4096.0)], axis=2)  # [BPC,T,128,XCOL]
        x66 = np.zeros((BPC, T, 128, 2, MAR + XCOL), E4M3)
        x66[:, :, :, 0, MAR:MAR + XCOL] = c8
        x66[:, :, :, 1, 0:XCOL] = c8
        m["xc66"] = x66
        x2 = np.zeros((BPC, T, 128, 2, 2 + XCOL), E4M3)
        x2[:, :, :, 0, 2:2 + XCOL] = c8
        x2[:, :, :, 1, 0:XCOL] = c8
        m["xc2"] = x2
        in_maps.append(m)
    return in_maps


def kernel(data, conv_w, conv_b, mlp_w1, mlp_w2):
    if "prog" not in _CACHED:
        _CACHED["prog"] = _build_program()
    nc = _CACHED["prog"]
    in_maps = make_in_maps(data, conv_w, conv_b, mlp_w1, mlp_w2)
    res = run_bass_kernel_spmd(nc, in_maps, list(range(N_CORES)))
    out = np.concatenate(
        [np.asarray(res.results[c]["spk"]).astype(np.float32)
         for c in range(N_CORES)], axis=0)
    return out.reshape(B, T, CH, H, W)


# revision 19
# speedup vs baseline: 1.0123x; 1.0093x over previous
"""Trainium2 Bass kernel for nn_ConvAttLIF (conv3x3 + temporal attention + LIF scan).

Sharding: data-parallel over batch B=16 across 8 NeuronCores (2 samples/core).

Layout: frames host-packed with shared row halos (33-wide rows: the right
halo of row r is the left halo of row r+1, both zero), so a frame is 1124
contiguous cols and the conv output span is 1056 cols = 3 psum chunks of 352.

Conv: per chunk, 15 f32r matmuls accumulate one psum bank:
  - 3 "pair" units (K=128): taps (-1,dx) and (+1,dx) fused by storing a
    second frame copy shifted 2 rows (66 cols) in partitions 64-127.
  - 3 "single" units (K=64): taps (0,dx) on partitions 0-63.
  - 9 "corr" units (K=128): [x_hi; x_lo] . [w_lo; w_hi] per tap, restoring
    ~fp32 accuracy from the 12-bit f32r operands (x_hi = trunc13(x)).
Chunks are processed in rotating order (frame f starts at chunk f%3) so each
frame's first psum bank was drained one chunk-stream earlier - no PE stall.

LIF scan: attention folded in via v_t = u_t/att_t, so each step is
v = g*c_t + y (STT), g = (v < thr_t)*v (STT, same engine - no cross-engine
hop in the serial chain), spike = (v >= thr_t) off-chain. The sample-1 tail
(no conv left to overlap) splits rows across DVE/Pool/ACT.

kernel(**inputs) takes the FULL unsharded inputs, returns the FULL output.
"""
import sys

sys.path.insert(0, "/opt/trn_rl_repo")

import numpy as np
import ml_dtypes
import concourse.bass as bass
import concourse.bacc as bacc
import concourse.tile as tile
import concourse.mybir as mybir
from concourse.bass_utils import run_bass_kernel_spmd

F32 = mybir.dt.float32
F32R = mybir.dt.float32r
FP8 = mybir.dt.float8e4
BF16 = mybir.dt.bfloat16
DR = mybir.MatmulPerfMode.DoubleRow
AF = mybir.ActivationFunctionType
OP = mybir.AluOpType
AX = mybir.AxisListType

B, T, CIN, H, W = 16, 20, 64, 32, 32
CH = 128
N_CORES = 8
BPC = B // N_CORES
ALPHA, VTH = 0.3, 0.6
HW = H * W                     # 1024
PW = W + 1                     # 33: row stride (shared halo col)
XCOL = 34 * PW + 2             # 1124 packed frame cols (+2 guard)
MAR = 2 * PW                   # 66: left margin in XA for the shifted copy
CN = 352                       # psum chunk cols (3 x 352 = 1056 out span)
OUT0 = PW + 1                  # 34: first out position in frame coords
NY = 25                        # y-tile ring size
TAPS = [(dy, dx) for dy in (-1, 0, 1) for dx in (-1, 0, 1)]


def _build_program():
    nc = bacc.Bacc("TRN2", target_bir_lowering=False, debug=False,
                   num_devices=N_CORES)

    xhi_d = nc.dram_tensor("xhi", [BPC, T, CIN, XCOL], F32,
                           kind="ExternalInput").ap()
    xc66_d = nc.dram_tensor("xc66", [BPC, T, 128, 2, MAR + XCOL], FP8,
                            kind="ExternalInput").ap()
    xc2_d = nc.dram_tensor("xc2", [BPC, T, 128, 2, 2 + XCOL], FP8,
                           kind="ExternalInput").ap()
    wpair_d = nc.dram_tensor("wpair", [128, 3 * 128], F32,
                             kind="ExternalInput").ap()
    wsing_d = nc.dram_tensor("wsing", [64, 3 * 128], F32,
                             kind="ExternalInput").ap()
    wc66_d = nc.dram_tensor("wc66", [128, 2, 3 * 128], FP8,
                            kind="ExternalInput").ap()
    wc2_d = nc.dram_tensor("wc2", [128, 2, 128], FP8,
                           kind="ExternalInput").ap()
    wc0_d = nc.dram_tensor("wc0", [128, 2, 128], FP8,
                           kind="ExternalInput").ap()
    bias_d = nc.dram_tensor("bias", [128, 1], F32, kind="ExternalInput").ap()
    w1t_d = nc.dram_tensor("w1t", [T, 5], F32, kind="ExternalInput").ap()
    w2t_d = nc.dram_tensor("w2t", [5, T], F32, kind="ExternalInput").ap()
    ident_d = nc.dram_tensor("ident", [128, 128], F32, kind="ExternalInput").ap()
    spk = nc.dram_tensor("spk", [BPC, T, CH, HW], FP8,
                         kind="ExternalOutput").ap()

    with tile.TileContext(nc) as tc:
        with tc.tile_pool(name="sb", bufs=1) as P1, \
             tc.tile_pool(name="scr", bufs=2) as P2, \
             tc.tile_pool(name="so", bufs=3) as P3, \
             tc.tile_pool(name="ps", bufs=1, space="PSUM") as PP:

            # ---- persistent tiles ----
            xas = [P1.tile([128, MAR + XCOL], F32R, tag=f"xa{i}", name=f"xa{i}")
                   for i in range(4)]
            xc66s = [P1.tile([128, 2, MAR + XCOL], FP8, tag=f"x6{i}",
                             name=f"x6{i}") for i in range(4)]
            xc2s = [P1.tile([128, 2, 2 + XCOL], FP8, tag=f"x2{i}",
                            name=f"x2{i}") for i in range(4)]

            def x_dma(s, t):
                f = s * T + t
                src = xhi_d[s, t].bitcast(F32R)
                nc.sync.dma_start(xas[f % 4][0:64, MAR:MAR + XCOL], src)
                nc.sync.dma_start(xas[f % 4][64:128, 0:XCOL], src)
                nc.sync.dma_start(xc66s[f % 4][:], xc66_d[s, t])
                nc.sync.dma_start(xc2s[f % 4][:], xc2_d[s, t])

            # startup order: frame-0 XA halves, pair/single weights (first
            # units of the first chunk), then the corr inputs
            f0src = xhi_d[0, 0].bitcast(F32R)
            nc.sync.dma_start(xas[0][0:64, MAR:MAR + XCOL], f0src)
            nc.sync.dma_start(xas[0][64:128, 0:XCOL], f0src)
            wpair = P1.tile([128, 3 * 128], F32R, tag="wpair", name="wpair")
            nc.sync.dma_start(wpair[:], wpair_d[:].bitcast(F32R))
            wsing = P1.tile([64, 3 * 128], F32R, tag="wsing", name="wsing")
            nc.sync.dma_start(wsing[:], wsing_d[:].bitcast(F32R))
            nc.sync.dma_start(xc66s[0][:], xc66_d[0, 0])
            nc.sync.dma_start(xc2s[0][:], xc2_d[0, 0])
            bias_t = P1.tile([128, 1], F32, tag="bias", name="bias")
            nc.sync.dma_start(bias_t[:], bias_d[:])

            wc66_s = P1.tile([128, 2, 3 * 128], FP8, tag="wc66", name="wc66")
            nc.sync.dma_start(wc66_s[:], wc66_d[:])
            wc2_s = P1.tile([128, 2, 128], FP8, tag="wc2", name="wc2")
            nc.sync.dma_start(wc2_s[:], wc2_d[:])
            wc0_s = P1.tile([128, 2, 128], FP8, tag="wc0", name="wc0")
            nc.sync.dma_start(wc0_s[:], wc0_d[:])
            w1t_s = P1.tile([T, 5], F32, tag="w1t", name="w1t")
            nc.sync.dma_start(w1t_s[:], w1t_d[:])
            w2t_s = P1.tile([5, T], F32, tag="w2t", name="w2t")
            nc.sync.dma_start(w2t_s[:], w2t_d[:])
            ident = P1.tile([128, 128], F32, tag="ident", name="ident")
            nc.sync.dma_start(ident[:], ident_d[:])
            ones_t = P1.tile([1, 128], F32, tag="ones", name="ones")
            nc.vector.memset(ones_t[:], 1.0)

            ys = [P1.tile([128, XCOL], F32, tag=f"y{i}", name=f"y{i}")
                  for i in range(NY)]
            gs = [P1.tile([128, HW], F32, tag=f"g{s}", name=f"g{s}")
                  for s in range(BPC)]
            # stats rows: 0-2 chunk sums, 3 -junk, 4 total, 5 max
            s_st = [P1.tile([128, 6 * T], F32, tag=f"S{s}", name=f"S{s}")
                    for s in range(BPC)]
            bc = [P1.tile([128, 4 * T], F32, tag=f"bc{s}", name=f"bc{s}")
                  for s in range(BPC)]

            engines = {"v": nc.vector, "p": nc.gpsimd}

            def conv_frame(s, t, skip_dma=False):
                f = s * T + t
                if not skip_dma:
                    x_dma(s, t)
                xa, x6, x2 = xas[f % 4], xc66s[f % 4], xc2s[f % 4]
                y = ys[f % NY]
                for ci in range(3):
                    c = (f + ci) % 3
                    o = OUT0 + CN * c
                    ps = PP.tile([128, CN], F32, tag=f"p{c}{f % 2}",
                                 name=f"p{c}{f % 2}")
                    units = []
                    for i, dx in enumerate((-1, 0, 1)):
                        units.append((wpair[:, i * 128:(i + 1) * 128],
                                      xa[0:128, MAR + o - PW + dx:
                                         MAR + o - PW + dx + CN], None))
                    for i, dx in enumerate((-1, 0, 1)):
                        units.append((wsing[:, i * 128:(i + 1) * 128],
                                      xa[0:64, MAR + o + dx:MAR + o + dx + CN],
                                      None))
                    # fp8 DoubleRow corr: plane0/plane1 pair taps (-1,dx)
                    # with (+1,dx) (delta 2*PW) and (0,-1) with (0,+1)
                    for i, dx in enumerate((-1, 0, 1)):
                        b0 = MAR + o - PW + dx
                        units.append((wc66_s[:, 0:2, i * 128:(i + 1) * 128],
                                      x6[:, 0:2, b0:b0 + CN], DR))
                    units.append((wc2_s[:, 0:2, :],
                                  x2[:, 0:2, 2 + o - 1:2 + o - 1 + CN], DR))
                    # tap (0,0) as DoubleRow with a zeroed second plane
                    units.append((wc0_s[:, 0:2, :],
                                  x6[:, 0:2, MAR + o:MAR + o + CN], DR))
                    for k, (w_ap, x_ap, pm) in enumerate(units):
                        nc.tensor.matmul(ps[:], w_ap, x_ap,
                                         start=(k == 0),
                                         stop=(k == len(units) - 1),
                                         perf_mode=pm)
                    nc.scalar.activation(
                        y[:, o:o + CN], ps[:], AF.Identity,
                        bias=bias_t[:, 0:1], scale=1.0 / 65536.0,
                        accum_out=s_st[s][:, c * T + t:c * T + t + 1])
                # stats: -junk sum, max over real cols, total
                yj = y[:, MAR:MAR + 32 * PW].rearrange(
                    "p (r c) -> p r c", c=PW)
                nc.vector.reduce_sum(s_st[s][:, 3 * T + t:3 * T + t + 1],
                                     yj[:, :, 0:1], axis=AX.XY, negate=True)
                ym = y[:, OUT0:OUT0 + 32 * PW].rearrange(
                    "p (r c) -> p r c", c=PW)
                nc.vector.reduce_max(s_st[s][:, 5 * T + t:5 * T + t + 1],
                                     ym[:, :, 0:W], axis=AX.XY)
                sv = s_st[s].rearrange("p (k t) -> p k t", t=T)
                nc.vector.reduce_sum(sv[:, 4:5, t:t + 1], sv[:, 0:4, t:t + 1],
                                     axis=AX.XY)

            def attention(s):
                S = s_st[s]
                psT1 = PP.tile([T, 128], F32, tag="pa0", name="psT1")
                nc.tensor.transpose(psT1[:], S[:, 4 * T:5 * T], ident[:])
                psT2 = PP.tile([T, 128], F32, tag="pa1", name="psT2")
                nc.tensor.transpose(psT2[:], S[:, 5 * T:6 * T], ident[:])
                tmp = P2.tile([T, 1], F32, tag="att_tmp", name="att_tmp")
                nc.vector.reduce_sum(tmp[:], psT1[:], axis=AX.X)
                att_in = P2.tile([T, 2], F32, tag="att_in", name="att_in")
                nc.vector.tensor_scalar_mul(att_in[:, 0:1], tmp[:],
                                            1.0 / (CH * HW))
                nc.vector.reduce_max(att_in[:, 1:2], psT2[:], axis=AX.X)
                ps5 = PP.tile([5, 2], F32, tag="pa0", name="ps5")
                nc.tensor.matmul(ps5[:], w1t_s[:], att_in[:], start=True,
                                 stop=True)
                h5 = P2.tile([5, 2], F32, tag="h5", name="h5")
                nc.vector.tensor_scalar_max(h5[:], ps5[:], 0.0)
                ps20 = PP.tile([T, 2], F32, tag="pa1", name="ps20")
                nc.tensor.matmul(ps20[:], w2t_s[:], h5[:], start=True, stop=True)
                a20 = P2.tile([T, 2], F32, tag="a20", name="a20")
                nc.vector.tensor_scalar_add(a20[:], ps20[:], 0.0)
                attp = P2.tile([T, 1], F32, tag="attp", name="attp")
                nc.vector.tensor_tensor(attp[:], a20[:, 0:1], a20[:, 1:2],
                                        op=OP.add)
                # sigmoid via exp + reciprocal (tighter than the Sigmoid table)
                expz = P2.tile([T, 1], F32, tag="expz", name="expz")
                nc.scalar.activation(expz[:], attp[:], AF.Exp, scale=-1.0)
                att1 = P2.tile([T, 1], F32, tag="att1", name="att1")
                nc.vector.tensor_scalar_add(att1[:], expz[:], 1.0)
                att = P2.tile([T, 1], F32, tag="att", name="att")
                nc.vector.reciprocal(att[:], att1[:])
                psT3 = PP.tile([1, T], F32, tag="pa0", name="psT3")
                nc.tensor.transpose(psT3[:], att[:, 0:1], ident[0:T, 0:T])
                atts = P2.tile([1, T + 1], F32, tag="atts", name="atts")
                nc.vector.tensor_scalar_add(atts[0:1, 1:T + 1], psT3[:], 0.0)
                nc.vector.tensor_scalar_add(atts[0:1, 0:1], psT3[0:1, 0:1],
                                            0.0)
                rec = P2.tile([1, T], F32, tag="rec", name="rec")
                nc.vector.reciprocal(rec[:], atts[0:1, 1:T + 1])
                rhs3 = P2.tile([1, 4 * T], F32, tag="rhs3", name="rhs3")
                nc.vector.scalar_tensor_tensor(
                    rhs3[0:1, 0:T], atts[0:1, 0:T], ALPHA, rec[:],
                    op0=OP.mult, op1=OP.mult)
                nc.vector.tensor_scalar_mul(rhs3[0:1, T:2 * T], rec[:], VTH)
                nc.vector.tensor_scalar_mul(rhs3[0:1, 2 * T:3 * T], rec[:],
                                            -VTH)
                nc.vector.tensor_scalar_mul(rhs3[0:1, 3 * T:4 * T], rec[:],
                                            -VTH * 1e8)
                ps_bc = PP.tile([128, 4 * T], F32, tag="pa1", name="ps_bc")
                nc.tensor.matmul(ps_bc[:], ones_t[:], rhs3[:], start=True,
                                 stop=True)
                nc.vector.tensor_scalar_add(bc[s][:], ps_bc[:], 0.0)

            def scan_step(s, t, vg, sp):
                f = s * T + t
                g = gs[s]
                if t == 0:
                    nc.vector.memset(g[:], 0.0)
                y = ys[f % NY]
                yv = y[:, OUT0:OUT0 + 32 * PW].rearrange(
                    "p (r c) -> p r c", c=PW)
                v = P2.tile([128, HW], F32, tag="v", name="v")
                m = (P2.tile([128, HW], F32, tag="m", name="m")
                     if any(e == "p" for e, _, _ in vg) else None)
                so = P3.tile([128, HW], FP8, tag="so", name="so")
                vv = v.rearrange("p (r c) -> p r c", c=W)
                gv = g.rearrange("p (r c) -> p r c", c=W)
                cb = bc[s][:, t:t + 1]
                tn = min(t + 1, T - 1)
                cbn = bc[s][:, tn:tn + 1]
                thr = bc[s][:, T + t:T + t + 1]
                nthr = bc[s][:, 2 * T + t:2 * T + t + 1]
                nthr8 = bc[s][:, 3 * T + t:3 * T + t + 1]
                for eng, r0, r1 in vg:
                    R = slice(r0 // W, r1 // W)
                    if eng == "v":
                        nc.vector.scalar_tensor_tensor(
                            vv[:, R, :], gv[:, R, :], cb, yv[:, R, 0:W],
                            op0=OP.mult, op1=OP.add)
                        nc.vector.scalar_tensor_tensor(
                            g[:, r0:r1], v[:, r0:r1], thr, v[:, r0:r1],
                            op0=OP.is_lt, op1=OP.mult)
                    else:
                        # Pool rows keep g pre-multiplied by c_{t+1}:
                        # v = g + y; m = (v<thr)*c_next; g = m*v
                        nc.gpsimd.tensor_tensor(
                            vv[:, R, :], gv[:, R, :], yv[:, R, 0:W],
                            op=OP.add)
                        nc.gpsimd.tensor_scalar(
                            m[:, r0:r1], v[:, r0:r1], thr, cbn,
                            op0=OP.is_lt, op1=OP.mult)
                        nc.gpsimd.tensor_tensor(
                            g[:, r0:r1], m[:, r0:r1], v[:, r0:r1],
                            op=OP.mult)
                for eng, r0, r1 in sp:
                    if eng == "sig":
                        # saturated sigmoid: 1e8*(v - thr) is past the f32
                        # sigmoid saturation point except ~1e-7 from thr
                        nc.scalar.activation(so[:, r0:r1], v[:, r0:r1],
                                             AF.Sigmoid, bias=nthr8,
                                             scale=1e8)
                    elif eng == "pm":
                        # spike from m (= (v<thr)*c_next): exactly 0 iff spike
                        nc.gpsimd.tensor_scalar(
                            so[:, r0:r1], m[:, r0:r1], 0.0, None,
                            op0=OP.is_equal)
                    else:
                        nc.vector.tensor_scalar(
                            so[:, r0:r1], v[:, r0:r1], thr, None,
                            op0=OP.is_ge)
                nc.sync.dma_start(spk[s, t], so[:])

            OVERLAP_VG = [("v", 0, 896), ("p", 896, HW)]
            OVERLAP_SP = [("sig", 0, HW)]
            TAIL_VG = [("v", 0, 384), ("v", 384, 768), ("p", 768, HW)]
            TAIL_SP = [("sig", 0, HW)]

            conv_frame(0, 0, skip_dma=True)
            for t in range(1, T):
                conv_frame(0, t)
            for t in range(4):
                conv_frame(1, t)
            # att(0) after 4 conv(1) frames: its PE ops sit behind queued
            # conv matmuls while the DVE/ACT chain resolves
            attention(0)
            # input prefetch 2 frames ahead: the spk DMA inside scan_step
            # waits on the scan result and blocks the SP queue behind it
            x_dma(1, 4)
            x_dma(1, 5)
            for t in range(T - 4):
                scan_step(0, t, OVERLAP_VG, OVERLAP_SP)
                conv_frame(1, t + 4, skip_dma=True)
                if t + 6 < T:
                    x_dma(1, t + 6)
            attention(1)
            for t in range(T - 4, T):
                scan_step(0, t, OVERLAP_VG, OVERLAP_SP)
            for t in range(T):
                scan_step(1, t, TAIL_VG, TAIL_SP)

    nc.compile()
    return nc


def _trunc13(a):
    # f32r = round-to-nearest, 11 explicit mantissa bits (HW-verified via
    # DMA roundtrip). Split values must be 11-bit so the hardware re-round
    # is a no-op and x_hi + x_lo == x exactly.
    u = np.ascontiguousarray(a, np.float32).view(np.uint32)
    r = (u + np.uint32(0x800)) & np.uint32(0xFFFFF000)
    return r.view(np.float32)


def _pad_frames(x):
    """[.., 64, 32, 32] -> [.., 64, XCOL] host-packed shared-halo frames."""
    lead = x.shape[:-2]
    padded = np.zeros(lead + (34, PW), np.float32)
    padded[..., 1:33, 1:33] = x
    out = np.zeros(lead + (XCOL,), np.float32)
    out[..., :34 * PW] = padded.reshape(lead + (34 * PW,))
    return out


E4M3 = ml_dtypes.float8_e4m3fn


def _fp8(a):
    return np.asarray(a, np.float32).astype(E4M3)


def _prep_host_inputs(conv_w, conv_b, mlp_w1, mlp_w2):
    wT = np.ascontiguousarray(np.transpose(conv_w, (1, 0, 2, 3)))  # [64,128,3,3]
    hi = {}
    c8 = {}
    for dy, dx in TAPS:
        blk = np.ascontiguousarray(wT[:, :, dy + 1, dx + 1])
        h = _trunc13(blk)
        lo = (blk - h).astype(np.float32)
        hi[(dy, dx)] = h
        # fp8 corr weights: [w_lo*2^16 ; w_hi*2^4] (psum scale 2^16 with
        # x_lo prescaled by 2^12 on the data side)
        c8[(dy, dx)] = np.concatenate(
            [_fp8(lo * 65536.0), _fp8(h * 16.0)], axis=0)          # [128,128]
    # T1 weights prescaled by 2^16 (exact) to share the corr psum scale
    wpair = np.concatenate(
        [np.concatenate([hi[(-1, dx)], hi[(1, dx)]], axis=0)
         for dx in (-1, 0, 1)], axis=1) * 65536.0                  # [128, 384]
    wsing = np.concatenate(
        [hi[(0, dx)] for dx in (-1, 0, 1)], axis=1) * 65536.0
    wc66 = np.stack(
        [np.concatenate([c8[(-1, dx)] for dx in (-1, 0, 1)], axis=1),
         np.concatenate([c8[(1, dx)] for dx in (-1, 0, 1)], axis=1)],
        axis=1)                                                    # [128,2,384]
    wc2 = np.stack([c8[(0, -1)], c8[(0, 1)]], axis=1)              # [128,2,128]
    return {
        "wpair": np.ascontiguousarray(wpair, np.float32),
        "wsing": np.ascontiguousarray(wsing, np.float32),
        "wc66": np.ascontiguousarray(wc66),
        "wc2": np.ascontiguousarray(wc2),
        "wc0": np.ascontiguousarray(
            np.stack([c8[(0, 0)], np.zeros_like(c8[(0, 0)])], axis=1)),
        "bias": np.ascontiguousarray(conv_b.reshape(128, 1), np.float32),
        "w1t": np.ascontiguousarray(mlp_w1.T).astype(np.float32),
        "w2t": np.ascontiguousarray(mlp_w2.T).astype(np.float32),
        "ident": np.eye(128, dtype=np.float32),
    }


_CACHED = {}


def make_in_maps(data, conv_w, conv_b, mlp_w1, mlp_w2):
    data = np.ascontiguousarray(data, np.float32)
    common = _prep_host_inputs(np.asarray(conv_w, np.float32),
                               np.asarray(conv_b, np.float32),
                               np.asarray(mlp_w1, np.float32),
                               np.asarray(mlp_w2, np.float32))
    in_maps = []
    for c in range(N_CORES):
        m = dict(common)
        shard = _pad_frames(data[c * BPC:(c + 1) * BPC])
        h = _trunc13(shard)
        m["xhi"] = h
        # fp8 corr data: [fp8(x_hi) ; fp8(x_lo*2^12)] in two shifted planes
        c8 = np.concatenate(
            [_fp8(h), _fp8((shard - h) * 4096.0)], axis=2)  # [BPC,T,128,XCOL]
        x66 = np.zeros((BPC, T, 128, 2, MAR + XCOL), E4M3)
        x66[:, :, :, 0, MAR:MAR + XCOL] = c8
        x66[:, :, :, 1, 0:XCOL] = c8
        m["xc66"] = x66
        x2 = np.zeros((BPC, T, 128, 2, 2 + XCOL), E4M3)
        x2[:, :, :, 0, 2:2 + XCOL] = c8
        x2[:, :, :, 1, 0:XCOL] = c8
        m["xc2"] = x2
        in_maps.append(m)
    return in_maps


def kernel(data, conv_w, conv_b, mlp_w1, mlp_w2):
    if "prog" not in _CACHED:
        _CACHED["prog"] = _build_program()
    nc = _CACHED["prog"]
    in_maps = make_in_maps(data, conv_w, conv_b, mlp_w1, mlp_w2)
    res = run_bass_kernel_spmd(nc, in_maps, list(range(N_CORES)))
    out = np.concatenate(
        [np.asarray(res.results[c]["spk"]).astype(np.float32)
         for c in range(N_CORES)], axis=0)
    return out.reshape(B, T, CH, H, W)
